# revision 2
# baseline (speedup 1.0000x reference)
"""Adaptive thresholding (11x11 box mean, BORDER_REPLICATE, THRESH_BINARY_INV)
on 8 TRN2 NeuronCores, data-parallel over the batch dim.

Algorithm per 128-row block of a 512x512 image:
  - DVE: prefix scan S of the edge-replicated row (x padded to 522 cols).
  - GpSimd: horizontal 11-tap sums W11[c] = S[c+10] - S[c-1], downcast f32r.
  - PE: PSUM = BM^T @ W11 (vertical band, weights 1.0) + BH^T @ halo rows
        + (-121*I)^T @ x (full f32), i.e. PSUM = 121*(mean - x).
  - DVE: out = (PSUM >= 242) * 255   [x <= mean-2  ->  255 else 0]
"""
import sys
sys.path.insert(0, '/opt/trn_rl_repo')
import numpy as np
import concourse.bass as bass
import concourse.tile as tile
from concourse import bacc, mybir
from concourse.bass_utils import run_bass_kernel_spmd

F32 = mybir.dt.float32
F32R = mybir.dt.float32r

N_CORES = 8
BATCH, H, W = 128, 512, 512
IMGS_PER_CORE = BATCH // N_CORES      # 16
ROWS_PER_CORE = IMGS_PER_CORE * H     # 8192
BLK = 128                             # rows per block
NBLK = H // BLK                       # 4 blocks per image
K = 11
PAD = K // 2                          # 5
WM = W + 2 * PAD                      # 522 scan input width
WS = WM + 1                           # 523 prefix width (leading zero col)


def _band_matrices():
    """Vertical band matrices, weights exactly 1.0 (fold /121 into compare)."""
    r = np.arange(BLK)
    bm_mid = (np.abs(r[:, None] - r[None, :]) <= PAD).astype(np.float32)
    bm_top = bm_mid.copy()
    for rr in range(PAD):
        bm_top[0, rr] += PAD - rr      # replicated rows above image top
    bm_bot = bm_mid.copy()
    for rr in range(BLK - PAD, BLK):
        bm_bot[BLK - 1, rr] += rr - (BLK - PAD - 1)

    # halo: partitions 0..4 = prev block rows (bstart-5..bstart-1),
    #       partitions 5..9 = next block rows (bstart+128..bstart+132)
    bh_mid = np.zeros((2 * PAD, BLK), dtype=np.float32)
    for p in range(PAD):
        bh_mid[p, 0:p + 1] = 1.0                  # k = p-5 -> r <= p
        bh_mid[PAD + p, BLK - PAD + p:BLK] = 1.0  # k = 128+p -> r >= 123+p
    bh_top = bh_mid.copy(); bh_top[0:PAD, :] = 0.0
    bh_bot = bh_mid.copy(); bh_bot[PAD:, :] = 0.0
    idn = (-121.0 * np.eye(BLK)).astype(np.float32)
    return bm_top, bm_mid, bm_bot, bh_top, bh_mid, bh_bot, idn


def _build():
    nc = bacc.Bacc(None, target_bir_lowering=False, debug=False)
    x_d = nc.declare_dram_parameter("x", [ROWS_PER_CORE, W], F32, isOutput=False)
    consts = {}
    for nm in ("bm_top", "bm_mid", "bm_bot"):
        consts[nm] = nc.declare_dram_parameter(nm, [BLK, BLK], F32R, isOutput=False)
    for nm in ("bh_top", "bh_mid", "bh_bot"):
        consts[nm] = nc.declare_dram_parameter(nm, [2 * PAD, BLK], F32R, isOutput=False)
    consts["idn"] = nc.declare_dram_parameter("idn", [BLK, BLK], F32, isOutput=False)
    out_d = nc.declare_dram_parameter("out", [ROWS_PER_CORE, W], F32, isOutput=True)

    with tile.TileContext(nc) as tc:
        with (
            tc.tile_pool(name="cpool", bufs=1) as cpool,
            tc.tile_pool(name="xm", bufs=3) as xm_pool,
            tc.tile_pool(name="scan", bufs=3) as s_pool,
            tc.tile_pool(name="w11", bufs=4) as w_pool,
            tc.tile_pool(name="halo", bufs=3) as h_pool,
            tc.tile_pool(name="outp", bufs=3) as o_pool,
            tc.tile_pool(name="psum", bufs=4, space=bass.MemorySpace.PSUM) as ps_pool,
        ):
            bm_t = {}
            for nm, d in consts.items():
                t = cpool.tile(list(d.shape), d.dtype, tag=nm)
                nc.sync.dma_start(t[:], d[:])
                bm_t[nm] = t

            # per-block state carried across the pipeline
            state = {}  # b -> dict(tiles)

            def stage_front(b):
                """steps 1-5: DMA in, margins, scan, zero col, W11."""
                i, pos = divmod(b, NBLK)
                r0 = i * H + pos * BLK
                xm = xm_pool.tile([BLK, WM], F32, tag="xm")
                nc.sync.dma_start(xm[:, PAD:PAD + W], x_d[r0:r0 + BLK, :])
                nc.scalar.copy(xm[:, 0:PAD],
                               xm[:, PAD:PAD + 1].to_broadcast((BLK, PAD)))
                nc.scalar.copy(xm[:, PAD + W:WM],
                               xm[:, PAD + W - 1:PAD + W].to_broadcast((BLK, PAD)))
                s = s_pool.tile([BLK, WS], F32, tag="scan")
                nc.vector.memset(s[:, 0:1], 0.0)
                nc.vector.tensor_tensor_scan(
                    s[:, 1:WS], xm[:, 0:WM], xm[:, 0:WM], 0.0,
                    op0=mybir.AluOpType.add, op1=mybir.AluOpType.bypass)
                w11 = w_pool.tile([BLK, W], F32R, tag="w11")
                nc.gpsimd.tensor_tensor(
                    w11[:], s[:, K:WS], s[:, 0:W], op=mybir.AluOpType.subtract)
                state[b] = {"xm": xm, "w11": w11}

            def stage_back(b):
                """steps 6-9: halo staging, matmuls, compare, DMA out."""
                i, pos = divmod(b, NBLK)
                r0 = i * H + pos * BLK
                st = state.pop(b)
                w11, xm = st["w11"], st["xm"]
                halo = h_pool.tile([2 * PAD, W], F32R, tag="halo")
                if pos > 0:
                    nc.sync.dma_start(halo[0:PAD, :], prev_w11[b - 1][BLK - PAD:BLK, :])
                else:
                    nc.sync.dma_start(halo[0:PAD, :], w11[0:PAD, :])  # zero weights
                if pos < NBLK - 1:
                    nc.sync.dma_start(halo[PAD:2 * PAD, :], state[b + 1]["w11"][0:PAD, :])
                else:
                    nc.sync.dma_start(halo[PAD:2 * PAD, :], w11[0:PAD, :])
                suffix = "top" if pos == 0 else ("bot" if pos == NBLK - 1 else "mid")
                ps = ps_pool.tile([BLK, W], F32, tag="ps")
                nc.tensor.matmul(ps[:], bm_t["bm_" + suffix][:], w11[:],
                                 start=True, stop=False)
                nc.tensor.matmul(ps[:], bm_t["bh_" + suffix][:], halo[:],
                                 start=False, stop=False)
                nc.tensor.matmul(ps[:], bm_t["idn"][:], xm[:, PAD:PAD + W],
                                 start=False, stop=True)
                ot = o_pool.tile([BLK, W], F32, tag="outp")
                nc.vector.tensor_scalar(
                    ot[:], ps[:], 242.0, 255.0,
                    op0=mybir.AluOpType.is_ge, op1=mybir.AluOpType.mult)
                nc.sync.dma_start(out_d[r0:r0 + BLK, :], ot[:])

            prev_w11 = {}
            for i in range(IMGS_PER_CORE):
                for pos in range(NBLK):
                    b = i * NBLK + pos
                    stage_front(b)
                    prev_w11[b] = state[b]["w11"]
                    if pos >= 1:
                        stage_back(b - 1)
                stage_back(i * NBLK + NBLK - 1)
                prev_w11.clear()
    nc.compile()
    return nc


_NC_CACHE = None


def kernel(x: np.ndarray) -> np.ndarray:
    global _NC_CACHE
    x = np.asarray(x, dtype=np.float32)
    xr = x.reshape(BATCH, H, W)

    bm_top, bm_mid, bm_bot, bh_top, bh_mid, bh_bot, idn = _band_matrices()
    if _NC_CACHE is None:
        _NC_CACHE = _build()
    nc = _NC_CACHE

    in_maps = []
    for c in range(N_CORES):
        shard = xr[c * IMGS_PER_CORE:(c + 1) * IMGS_PER_CORE].reshape(
            ROWS_PER_CORE, W)
        in_maps.append({
            "x": np.ascontiguousarray(shard),
            "bm_top": bm_top, "bm_mid": bm_mid, "bm_bot": bm_bot,
            "bh_top": bh_top, "bh_mid": bh_mid, "bh_bot": bh_bot,
            "idn": idn,
        })
    res = run_bass_kernel_spmd(nc, in_maps, core_ids=list(range(N_CORES)))
    out = np.empty((BATCH, H, W), dtype=np.float32)
    for c in range(N_CORES):
        out[c * IMGS_PER_CORE:(c + 1) * IMGS_PER_CORE] = \
            res.results[c]["out"].reshape(IMGS_PER_CORE, H, W)
    return out.reshape(BATCH, H, W, 1)


# revision 6
# speedup vs baseline: 2.5133x; 2.5133x over previous
"""Adaptive thresholding (11x11 box mean, BORDER_REPLICATE, THRESH_BINARY_INV)
on 8 TRN2 NeuronCores, data-parallel over the batch dim.

V3 design, per 128-row block of a 512x512 image (fp16 data path):
  - x DMA'd as fp16. Block tile layout [128, 533]: cols 0..10 zeros,
    11..15 left margin (edge replicate), 16..527 x, 528..532 right margin.
  - DVE sliding-window scan: state = (xp[t] + state) - pad[t-11], one op
    -> W11 (horizontal 11-tap sums) in fp16 at cols 10..521 of out_scr.
  - PE (all fp16, N=512): PSUM = BM^T@W11 + BH^T@halo + (-121*I)^T@x
    = 121*(mean - x).
  - ACT: Sign(PSUM - 242) -> bf16 {-1,0,+1}.
Host: out = (sign >= 0) * 255  (inclusive compare matches x <= mean-2).
"""
import sys
sys.path.insert(0, '/opt/trn_rl_repo')
import numpy as np
import concourse.bass as bass
import concourse.tile as tile
from concourse import bacc, mybir
from concourse.bass_utils import run_bass_kernel_spmd

F32 = mybir.dt.float32
F16 = mybir.dt.float16
BF16 = mybir.dt.bfloat16

N_CORES = 8
BATCH, H, W = 128, 512, 512
IMGS_PER_CORE = BATCH // N_CORES      # 16
ROWS_PER_CORE = IMGS_PER_CORE * H     # 8192
BLK = 128
NBLK = H // BLK                       # 4
K = 11
PAD = K // 2                          # 5
ZH = K                                # zero head width (11)
WT = ZH + PAD + W + PAD               # 533 block tile width
XP0 = ZH                              # xp starts at col 11
X0 = ZH + PAD                         # x starts at col 16
WSC = PAD + W + PAD                   # 522 scan length
W11OFF = K - 1                        # W11[c] = out_scr[:, 10 + c]


def _band_matrices(dtype=np.float16):
    r = np.arange(BLK)
    bm_mid = (np.abs(r[:, None] - r[None, :]) <= PAD).astype(dtype)
    bm_top = bm_mid.copy()
    for rr in range(PAD):
        bm_top[0, rr] += dtype(PAD - rr)
    bm_bot = bm_mid.copy()
    for rr in range(BLK - PAD, BLK):
        bm_bot[BLK - 1, rr] += dtype(rr - (BLK - PAD - 1))
    bh_mid = np.zeros((2 * PAD, BLK), dtype=dtype)
    for p in range(PAD):
        bh_mid[p, 0:p + 1] = 1.0
        bh_mid[PAD + p, BLK - PAD + p:BLK] = 1.0
    bh_top = bh_mid.copy(); bh_top[0:PAD, :] = 0.0
    bh_bot = bh_mid.copy(); bh_bot[PAD:, :] = 0.0
    idn = (-121.0 * np.eye(BLK)).astype(dtype)
    return {"bm_top": bm_top, "bm_mid": bm_mid, "bm_bot": bm_bot,
            "bh_top": bh_top, "bh_mid": bh_mid, "bh_bot": bh_bot,
            "idn": idn}


def _build():
    nc = bacc.Bacc(None, target_bir_lowering=False, debug=False)
    x_d = nc.declare_dram_parameter("x", [ROWS_PER_CORE, W], F16, isOutput=False)
    consts = {}
    for nm in ("bm_top", "bm_mid", "bm_bot", "idn"):
        consts[nm] = nc.declare_dram_parameter(nm, [BLK, BLK], F16, isOutput=False)
    for nm in ("bh_top", "bh_mid", "bh_bot"):
        consts[nm] = nc.declare_dram_parameter(nm, [2 * PAD, BLK], F16, isOutput=False)
    out_d = nc.declare_dram_parameter("out", [ROWS_PER_CORE, W], BF16, isOutput=True)
    xv = x_d[:].rearrange("(i p q) c -> i p q c", p=NBLK, q=BLK)   # [16,4,128,512]
    ov = out_d[:].rearrange("(i p q) c -> i p q c", p=NBLK, q=BLK)

    with tile.TileContext(nc) as tc:
        with (
            tc.tile_pool(name="cpool", bufs=1) as cpool,
            tc.tile_pool(name="xin", bufs=2) as x_pool,
            tc.tile_pool(name="scr", bufs=4) as s_pool,
            tc.tile_pool(name="halo", bufs=3) as h_pool,
            tc.tile_pool(name="outp", bufs=2) as o_pool,
            tc.tile_pool(name="psum", bufs=4, space=bass.MemorySpace.PSUM) as ps_pool,
        ):
            ct = {}
            for nm, d in consts.items():
                t = cpool.tile(list(d.shape), F16, tag=nm)
                nc.sync.dma_start(t[:], d[:])
                ct[nm] = t
            bias_t = cpool.tile([BLK, 1], F32, tag="bias")
            nc.vector.memset(bias_t[:], -242.0)

            scr = {}       # block b -> out_scr tile
            ximg_of = {}   # image -> (ximg, oimg)

            def front_img(i):
                ximg = x_pool.tile([BLK, NBLK, WT], F16, tag="ximg")
                nc.sync.dma_start(
                    ximg[:, :, X0:X0 + W],
                    xv[i].rearrange("p q c -> q p c"))
                nc.gpsimd.memset(ximg[:, :, 0:ZH], 0.0)
                nc.scalar.copy(ximg[:, :, XP0:X0],
                               ximg[:, :, X0:X0 + 1].to_broadcast((BLK, NBLK, PAD)))
                nc.scalar.copy(ximg[:, :, X0 + W:WT],
                               ximg[:, :, X0 + W - 1:X0 + W].to_broadcast(
                                   (BLK, NBLK, PAD)))
                oimg = o_pool.tile([BLK, NBLK, W], BF16, tag="oimg")
                ximg_of[i] = (ximg, oimg)

            def front_blk(b):
                i, pos = divmod(b, NBLK)
                ximg, _ = ximg_of[i]
                s = s_pool.tile([BLK, WSC], F16, tag="scr")
                nc.vector.tensor_tensor_scan(
                    s[:], ximg[:, pos, XP0:WT], ximg[:, pos, 0:WSC], 0.0,
                    op0=mybir.AluOpType.add, op1=mybir.AluOpType.subtract)
                scr[b] = s

            def back_blk(b):
                i, pos = divmod(b, NBLK)
                ximg, oimg = ximg_of[i]
                s = scr[b]
                halo = h_pool.tile([2 * PAD, W], F16, tag="halo")
                if pos > 0:
                    src_prev = scr[b - 1][BLK - PAD:BLK, W11OFF:W11OFF + W]
                else:
                    src_prev = s[0:PAD, W11OFF:W11OFF + W]   # zero weights
                if pos < NBLK - 1:
                    src_next = scr[b + 1][0:PAD, W11OFF:W11OFF + W]
                else:
                    src_next = s[0:PAD, W11OFF:W11OFF + W]   # zero weights
                nc.gpsimd.dma_start(halo[0:PAD, :], src_prev)
                nc.gpsimd.dma_start(halo[PAD:2 * PAD, :], src_next)
                sfx = "top" if pos == 0 else ("bot" if pos == NBLK - 1 else "mid")
                ps = ps_pool.tile([BLK, W], F32, tag="ps")
                nc.tensor.matmul(ps[:], ct["bm_" + sfx][:],
                                 s[:, W11OFF:W11OFF + W], start=True, stop=False)
                nc.tensor.matmul(ps[:], ct["bh_" + sfx][:], halo[:],
                                 start=False, stop=False)
                nc.tensor.matmul(ps[:], ct["idn"][:], ximg[:, pos, X0:X0 + W],
                                 start=False, stop=True)
                nc.scalar.activation(
                    oimg[:, pos, :], ps[:], mybir.ActivationFunctionType.Sign,
                    bias=bias_t[:], scale=1.0)

            def flush_img(i):
                _, oimg = ximg_of.pop(i)
                nc.sync.dma_start(ov[i].rearrange("p q c -> q p c"), oimg[:])
                for pos in range(NBLK):
                    scr.pop(i * NBLK + pos, None)

            for i in range(IMGS_PER_CORE):
                front_img(i)
                for pos in range(NBLK):
                    b = i * NBLK + pos
                    front_blk(b)
                    if pos >= 1:
                        back_blk(b - 1)
                back_blk(i * NBLK + NBLK - 1)
                flush_img(i)
    nc.compile()
    return nc


_NC_CACHE = None


def kernel(x: np.ndarray) -> np.ndarray:
    global _NC_CACHE
    x = np.asarray(x, dtype=np.float32)
    x16 = x.reshape(BATCH, H, W).astype(np.float16)

    consts = _band_matrices()
    if _NC_CACHE is None:
        _NC_CACHE = _build()
    nc = _NC_CACHE

    in_maps = []
    for c in range(N_CORES):
        shard = x16[c * IMGS_PER_CORE:(c + 1) * IMGS_PER_CORE].reshape(
            ROWS_PER_CORE, W)
        m = {"x": np.ascontiguousarray(shard)}
        m.update(consts)
        in_maps.append(m)
    res = run_bass_kernel_spmd(nc, in_maps, core_ids=list(range(N_CORES)))
    out = np.empty((BATCH, H, W), dtype=np.float32)
    for c in range(N_CORES):
        sgn = res.results[c]["out"].astype(np.float32)
        out[c * IMGS_PER_CORE:(c + 1) * IMGS_PER_CORE] = \
            ((sgn >= 0.0) * np.float32(255.0)).reshape(IMGS_PER_CORE, H, W)
    return out.reshape(BATCH, H, W, 1)


# revision 10
# speedup vs baseline: 3.1158x; 1.2397x over previous
"""Adaptive thresholding (11x11 box mean, BORDER_REPLICATE, THRESH_BINARY_INV)
on 8 TRN2 NeuronCores, data-parallel over the batch dim.

V4 design, per 512x512 image (fp16 data path), 4 row-blocks of 128:
  - x DMA'd as fp16 into ximg [128, 4, 533]: per segment, cols 0..10 zeros,
    11..15 left margin, 16..527 x rows, 528..532 right margin.
  - ONE DVE sliding-window scan over the flattened [128, 2121] view:
    state = (xp[t] + state) - xp[t-11]; the 11-col zero head between
    segments self-drains the window state, so segments stay independent.
    Output (fp16) holds the horizontal 11-tap sums W11 per segment.
  - PE per block (all fp16, N=512, one PSUM bank):
      main band  BM^T @ W11_seg          (K=128)
      identity   (-121*I)^T @ x_seg      (K=128)
      halo prev  BHP^T @ W11_prevseg[96:128]  (K=32, tile_position (96,0))
      halo next  BHN^T @ W11_nextseg[0:32]    (K=32, tile_position (0,0))
    PSUM = 121*(mean - x).
  - ACT: Sign(PSUM - 242) -> bf16 {-1,0,+1}.
Host: out = (sign >= 0) * 255  (inclusive compare matches x <= mean-2).
"""
import sys
sys.path.insert(0, '/opt/trn_rl_repo')
import numpy as np
import concourse.bass as bass
import concourse.tile as tile
from concourse import bacc, mybir
from concourse.bass_utils import run_bass_kernel_spmd

F32 = mybir.dt.float32
F16 = mybir.dt.float16
BF16 = mybir.dt.bfloat16

N_CORES = 8
BATCH, H, W = 128, 512, 512
IMGS_PER_CORE = BATCH // N_CORES      # 16
ROWS_PER_CORE = IMGS_PER_CORE * H     # 8192
BLK = 128
NBLK = H // BLK                       # 4
K = 11
PAD = K // 2                          # 5
ZH = K                                # zero head width
WT = ZH + PAD + W + PAD               # 533 segment width
XP0 = ZH                              # xp offset within segment (11)
X0 = ZH + PAD                         # x offset within segment (16)
FLAT = NBLK * WT                      # 2132
SCLEN = FLAT - ZH                     # 2121 scan steps
KH = 32                               # halo row-group size


def _band_matrices(dtype=np.float16):
    r = np.arange(BLK)
    bm_mid = (np.abs(r[:, None] - r[None, :]) <= PAD).astype(dtype)
    bm_top = bm_mid.copy()
    for rr in range(PAD):
        bm_top[0, rr] += dtype(PAD - rr)
    bm_bot = bm_mid.copy()
    for rr in range(BLK - PAD, BLK):
        bm_bot[BLK - 1, rr] += dtype(rr - (BLK - PAD - 1))
    # halo prev: weight rows are prev-segment partitions 96..127 (rel 0..31);
    # partition 96+p is image row (seg base - 32 + p); nonzero for p>=27:
    # row k = -32+p affects output r iff |r - k| <= 5 -> r <= p - 27.
    bhp = np.zeros((BLK, BLK), dtype=dtype)
    for p in range(BLK - PAD, BLK):
        bhp[p, 0:p - (BLK - PAD) + 1] = 1.0
    # halo next: partitions 0..31 of next segment = image rows 128+p;
    # affects r iff r >= 123+p, for p in 0..4.
    bhn = np.zeros((BLK, BLK), dtype=dtype)
    for p in range(PAD):
        bhn[p, BLK - PAD + p:BLK] = 1.0
    idn = (-121.0 * np.eye(BLK)).astype(dtype)
    return {"bm_top": bm_top, "bm_mid": bm_mid, "bm_bot": bm_bot,
            "bhp": bhp, "bhn": bhn, "idn": idn}


def _build():
    nc = bacc.Bacc(None, target_bir_lowering=False, debug=False)
    x_d = nc.declare_dram_parameter("x", [ROWS_PER_CORE, W], F16, isOutput=False)
    shapes = {"bm_top": [BLK, BLK], "bm_mid": [BLK, BLK], "bm_bot": [BLK, BLK],
              "bhp": [BLK, BLK], "bhn": [BLK, BLK], "idn": [BLK, BLK]}
    consts = {nm: nc.declare_dram_parameter(nm, sh, F16, isOutput=False)
              for nm, sh in shapes.items()}
    out_d = nc.declare_dram_parameter("out", [ROWS_PER_CORE, W], BF16, isOutput=True)
    xv = x_d[:].rearrange("(i p q) c -> i p q c", p=NBLK, q=BLK)   # [16,4,128,512]
    ov = out_d[:].rearrange("(i p q) c -> i p q c", p=NBLK, q=BLK)

    with tile.TileContext(nc) as tc:
        with (
            tc.tile_pool(name="cpool", bufs=1) as cpool,
            tc.tile_pool(name="xin", bufs=2) as x_pool,
            tc.tile_pool(name="scr", bufs=2) as s_pool,
            tc.tile_pool(name="outp", bufs=2) as o_pool,
            tc.tile_pool(name="psum", bufs=4, space=bass.MemorySpace.PSUM) as ps_pool,
        ):
            ct = {}
            for nm, d in consts.items():
                t = cpool.tile(list(d.shape), F16, tag=nm)
                nc.sync.dma_start(t[:], d[:])
                ct[nm] = t
            bias_t = cpool.tile([BLK, 1], F32, tag="bias")
            nc.vector.memset(bias_t[:], -242.0)

            imgs = {}  # i -> (ximg, scr_img, oimg)

            def front_img(i):
                ximg = x_pool.tile([BLK, NBLK, WT], F16, tag="ximg")
                nc.sync.dma_start(
                    ximg[:, :, X0:X0 + W],
                    xv[i].rearrange("p q c -> q p c"))
                nc.gpsimd.memset(ximg[:, :, 0:ZH], 0.0)
                nc.scalar.copy(ximg[:, :, XP0:X0],
                               ximg[:, :, X0:X0 + 1].to_broadcast((BLK, NBLK, PAD)))
                nc.scalar.copy(ximg[:, :, X0 + W:WT],
                               ximg[:, :, X0 + W - 1:X0 + W].to_broadcast(
                                   (BLK, NBLK, PAD)))
                flat = ximg[:].rearrange("q p c -> q (p c)")
                s = s_pool.tile([BLK, SCLEN], F16, tag="scr")
                nc.vector.tensor_tensor_scan(
                    s[:], flat[:, ZH:FLAT], flat[:, 0:SCLEN], 0.0,
                    op0=mybir.AluOpType.add, op1=mybir.AluOpType.subtract)
                oimg = o_pool.tile([BLK, NBLK, W], BF16, tag="oimg")
                imgs[i] = (ximg, s, oimg)

            def back_img(i):
                ximg, s, oimg = imgs.pop(i)
                for pos in range(NBLK):
                    # W11 for segment pos lives at s[:, pos*WT+10 : +512]
                    seg = pos * WT + (K - 1)
                    sfx = "top" if pos == 0 else ("bot" if pos == NBLK - 1 else "mid")
                    ps = ps_pool.tile([BLK, W], F32, tag="ps")
                    mms = [(ct["bm_" + sfx][:], s[:, seg:seg + W], None)]
                    mms.append((ct["idn"][:], ximg[:, pos, X0:X0 + W], None))
                    if pos > 0:
                        pseg = (pos - 1) * WT + (K - 1)
                        mms.append((ct["bhp"][:], s[:, pseg:pseg + W], None))
                    if pos < NBLK - 1:
                        nseg = (pos + 1) * WT + (K - 1)
                        mms.append((ct["bhn"][:], s[:, nseg:nseg + W], None))
                    for j, (lhsT, rhs, tp) in enumerate(mms):
                        nc.tensor.matmul(ps[:], lhsT, rhs,
                                         start=(j == 0), stop=(j == len(mms) - 1),
                                         tile_position=tp)
                    nc.scalar.activation(
                        oimg[:, pos, :], ps[:], mybir.ActivationFunctionType.Sign,
                        bias=bias_t[:], scale=1.0)
                nc.sync.dma_start(ov[i].rearrange("p q c -> q p c"), oimg[:])

            front_img(0)
            for i in range(IMGS_PER_CORE):
                if i + 1 < IMGS_PER_CORE:
                    front_img(i + 1)
                back_img(i)
    nc.compile()
    return nc


_NC_CACHE = None


def kernel(x: np.ndarray) -> np.ndarray:
    global _NC_CACHE
    x = np.asarray(x, dtype=np.float32)
    x16 = x.reshape(BATCH, H, W).astype(np.float16)

    consts = _band_matrices()
    if _NC_CACHE is None:
        _NC_CACHE = _build()
    nc = _NC_CACHE

    in_maps = []
    for c in range(N_CORES):
        shard = x16[c * IMGS_PER_CORE:(c + 1) * IMGS_PER_CORE].reshape(
            ROWS_PER_CORE, W)
        m = {"x": np.ascontiguousarray(shard)}
        m.update(consts)
        in_maps.append(m)
    res = run_bass_kernel_spmd(nc, in_maps, core_ids=list(range(N_CORES)))
    out = np.empty((BATCH, H, W), dtype=np.float32)
    for c in range(N_CORES):
        sgn = res.results[c]["out"].astype(np.float32)
        out[c * IMGS_PER_CORE:(c + 1) * IMGS_PER_CORE] = \
            ((sgn >= 0.0) * np.float32(255.0)).reshape(IMGS_PER_CORE, H, W)
    return out.reshape(BATCH, H, W, 1)


# revision 11
# speedup vs baseline: 4.6271x; 1.4850x over previous
"""Adaptive thresholding (11x11 box mean, BORDER_REPLICATE, THRESH_BINARY_INV)
on 8 TRN2 NeuronCores, data-parallel over the batch dim.

V4 design, per 512x512 image (fp16 data path), 4 row-blocks of 128:
  - x DMA'd as fp16 into ximg [128, 4, 533]: per segment, cols 0..10 zeros,
    11..15 left margin, 16..527 x rows, 528..532 right margin.
  - ONE DVE sliding-window scan over the flattened [128, 2121] view:
    state = (xp[t] + state) - xp[t-11]; the 11-col zero head between
    segments self-drains the window state, so segments stay independent.
    Output (fp16) holds the horizontal 11-tap sums W11 per segment.
  - PE per block (all fp16, N=512, one PSUM bank):
      main band  BM^T @ W11_seg          (K=128)
      identity   (-121*I)^T @ x_seg      (K=128)
      halo prev  BHP^T @ W11_prevseg[96:128]  (K=32, tile_position (96,0))
      halo next  BHN^T @ W11_nextseg[0:32]    (K=32, tile_position (0,0))
    PSUM = 121*(mean - x).
  - ACT: Sign(PSUM - 242) -> bf16 {-1,0,+1}.
Host: out = (sign >= 0) * 255  (inclusive compare matches x <= mean-2).
"""
import sys
sys.path.insert(0, '/opt/trn_rl_repo')
import numpy as np
import concourse.bass as bass
import concourse.tile as tile
from concourse import bacc, mybir
from concourse.bass_utils import run_bass_kernel_spmd

F32 = mybir.dt.float32
F16 = mybir.dt.float16
BF16 = mybir.dt.bfloat16

N_CORES = 8
BATCH, H, W = 128, 512, 512
IMGS_PER_CORE = BATCH // N_CORES      # 16
ROWS_PER_CORE = IMGS_PER_CORE * H     # 8192
BLK = 128
NBLK = H // BLK                       # 4
K = 11
PAD = K // 2                          # 5
ZH = K                                # zero head width
WT = ZH + PAD + W + PAD               # 533 segment width
XP0 = ZH                              # xp offset within segment (11)
X0 = ZH + PAD                         # x offset within segment (16)
FLAT = NBLK * WT                      # 2132
SCLEN = FLAT - ZH                     # 2121 scan steps
KH = 32                               # halo row-group size


def _band_matrices(dtype=np.float16):
    r = np.arange(BLK)
    bm_mid = (np.abs(r[:, None] - r[None, :]) <= PAD).astype(dtype)
    bm_top = bm_mid.copy()
    for rr in range(PAD):
        bm_top[0, rr] += dtype(PAD - rr)
    bm_bot = bm_mid.copy()
    for rr in range(BLK - PAD, BLK):
        bm_bot[BLK - 1, rr] += dtype(rr - (BLK - PAD - 1))
    # halo prev: weight rows are prev-segment partitions 96..127 (rel 0..31);
    # partition 96+p is image row (seg base - 32 + p); nonzero for p>=27:
    # row k = -32+p affects output r iff |r - k| <= 5 -> r <= p - 27.
    bhp = np.zeros((BLK, BLK), dtype=dtype)
    for p in range(BLK - PAD, BLK):
        bhp[p, 0:p - (BLK - PAD) + 1] = 1.0
    # halo next: partitions 0..31 of next segment = image rows 128+p;
    # affects r iff r >= 123+p, for p in 0..4.
    bhn = np.zeros((BLK, BLK), dtype=dtype)
    for p in range(PAD):
        bhn[p, BLK - PAD + p:BLK] = 1.0
    idn = (-121.0 * np.eye(BLK)).astype(dtype)
    return {"bm_top": bm_top, "bm_mid": bm_mid, "bm_bot": bm_bot,
            "bhp": bhp, "bhn": bhn, "idn": idn}


def _build():
    nc = bacc.Bacc(None, target_bir_lowering=False, debug=False)
    x_d = nc.declare_dram_parameter("x", [ROWS_PER_CORE, W], F16, isOutput=False)
    shapes = {"bm_top": [BLK, BLK], "bm_mid": [BLK, BLK], "bm_bot": [BLK, BLK],
              "bhp": [BLK, BLK], "bhn": [BLK, BLK], "idn": [BLK, BLK]}
    consts = {nm: nc.declare_dram_parameter(nm, sh, F16, isOutput=False)
              for nm, sh in shapes.items()}
    out_d = nc.declare_dram_parameter("out", [ROWS_PER_CORE, W], BF16, isOutput=True)
    xv = x_d[:].rearrange("(i p q) c -> i p q c", p=NBLK, q=BLK)   # [16,4,128,512]
    ov = out_d[:].rearrange("(i p q) c -> i p q c", p=NBLK, q=BLK)

    with tile.TileContext(nc) as tc:
        with (
            tc.tile_pool(name="cpool", bufs=1) as cpool,
            tc.tile_pool(name="xin", bufs=3) as x_pool,
            tc.tile_pool(name="scr", bufs=3) as s_pool,
            tc.tile_pool(name="outp", bufs=3) as o_pool,
            tc.tile_pool(name="psum", bufs=4, space=bass.MemorySpace.PSUM) as ps_pool,
        ):
            ct = {}
            for nm, d in consts.items():
                t = cpool.tile(list(d.shape), F16, tag=nm)
                nc.sync.dma_start(t[:], d[:])
                ct[nm] = t
            bias_t = cpool.tile([BLK, 1], F32, tag="bias")
            nc.vector.memset(bias_t[:], -242.0)

            imgs = {}  # i -> (ximg, scr_img, oimg)

            def front_img(i):
                ximg = x_pool.tile([BLK, NBLK, WT], F16, tag="ximg")
                nc.sync.dma_start(
                    ximg[:, :, X0:X0 + W],
                    xv[i].rearrange("p q c -> q p c"))
                nc.gpsimd.memset(ximg[:, :, 0:ZH], 0.0)
                nc.scalar.copy(ximg[:, :, XP0:X0],
                               ximg[:, :, X0:X0 + 1].to_broadcast((BLK, NBLK, PAD)))
                nc.scalar.copy(ximg[:, :, X0 + W:WT],
                               ximg[:, :, X0 + W - 1:X0 + W].to_broadcast(
                                   (BLK, NBLK, PAD)))
                flat = ximg[:].rearrange("q p c -> q (p c)")
                s = s_pool.tile([BLK, SCLEN], F16, tag="scr")
                nc.vector.tensor_tensor_scan(
                    s[:], flat[:, ZH:FLAT], flat[:, 0:SCLEN], 0.0,
                    op0=mybir.AluOpType.add, op1=mybir.AluOpType.subtract)
                oimg = o_pool.tile([BLK, NBLK, W], BF16, tag="oimg")
                imgs[i] = (ximg, s, oimg)

            def back_img(i):
                ximg, s, oimg = imgs.pop(i)
                for pos in range(NBLK):
                    # W11 for segment pos lives at s[:, pos*WT+10 : +512]
                    seg = pos * WT + (K - 1)
                    sfx = "top" if pos == 0 else ("bot" if pos == NBLK - 1 else "mid")
                    ps = ps_pool.tile([BLK, W], F32, tag="ps")
                    mms = [(ct["bm_" + sfx][:], s[:, seg:seg + W], None)]
                    mms.append((ct["idn"][:], ximg[:, pos, X0:X0 + W], None))
                    if pos > 0:
                        pseg = (pos - 1) * WT + (K - 1)
                        mms.append((ct["bhp"][:], s[:, pseg:pseg + W], None))
                    if pos < NBLK - 1:
                        nseg = (pos + 1) * WT + (K - 1)
                        mms.append((ct["bhn"][:], s[:, nseg:nseg + W], None))
                    for j, (lhsT, rhs, tp) in enumerate(mms):
                        nc.tensor.matmul(ps[:], lhsT, rhs,
                                         start=(j == 0), stop=(j == len(mms) - 1),
                                         tile_position=tp)
                    nc.scalar.activation(
                        oimg[:, pos, :], ps[:], mybir.ActivationFunctionType.Sign,
                        bias=bias_t[:], scale=1.0)
                nc.sync.dma_start(ov[i].rearrange("p q c -> q p c"), oimg[:])

            front_img(0)
            front_img(1)
            for i in range(IMGS_PER_CORE):
                if i + 2 < IMGS_PER_CORE:
                    front_img(i + 2)
                back_img(i)
    nc.compile()
    return nc


_NC_CACHE = None


def kernel(x: np.ndarray) -> np.ndarray:
    global _NC_CACHE
    x = np.asarray(x, dtype=np.float32)
    x16 = x.reshape(BATCH, H, W).astype(np.float16)

    consts = _band_matrices()
    if _NC_CACHE is None:
        _NC_CACHE = _build()
    nc = _NC_CACHE

    in_maps = []
    for c in range(N_CORES):
        shard = x16[c * IMGS_PER_CORE:(c + 1) * IMGS_PER_CORE].reshape(
            ROWS_PER_CORE, W)
        m = {"x": np.ascontiguousarray(shard)}
        m.update(consts)
        in_maps.append(m)
    res = run_bass_kernel_spmd(nc, in_maps, core_ids=list(range(N_CORES)))
    out = np.empty((BATCH, H, W), dtype=np.float32)
    for c in range(N_CORES):
        sgn = res.results[c]["out"].astype(np.float32)
        out[c * IMGS_PER_CORE:(c + 1) * IMGS_PER_CORE] = \
            ((sgn >= 0.0) * np.float32(255.0)).reshape(IMGS_PER_CORE, H, W)
    return out.reshape(BATCH, H, W, 1)


# revision 12
# speedup vs baseline: 4.8955x; 1.0580x over previous
"""Adaptive thresholding (11x11 box mean, BORDER_REPLICATE, THRESH_BINARY_INV)
on 8 TRN2 NeuronCores, data-parallel over the batch dim.

V4 design, per 512x512 image (fp16 data path), 4 row-blocks of 128:
  - x DMA'd as fp16 into ximg [128, 4, 533]: per segment, cols 0..10 zeros,
    11..15 left margin, 16..527 x rows, 528..532 right margin.
  - ONE DVE sliding-window scan over the flattened [128, 2121] view:
    state = (xp[t] + state) - xp[t-11]; the 11-col zero head between
    segments self-drains the window state, so segments stay independent.
    Output (fp16) holds the horizontal 11-tap sums W11 per segment.
  - PE per block (all fp16, N=512, one PSUM bank):
      main band  BM^T @ W11_seg          (K=128)
      identity   (-121*I)^T @ x_seg      (K=128)
      halo prev  BHP^T @ W11_prevseg[96:128]  (K=32, tile_position (96,0))
      halo next  BHN^T @ W11_nextseg[0:32]    (K=32, tile_position (0,0))
    PSUM = 121*(mean - x).
  - ACT: Sign(PSUM - 242) -> bf16 {-1,0,+1}.
Host: out = (sign >= 0) * 255  (inclusive compare matches x <= mean-2).
"""
import sys
sys.path.insert(0, '/opt/trn_rl_repo')
import numpy as np
import concourse.bass as bass
import concourse.tile as tile
from concourse import bacc, mybir
from concourse.bass_utils import run_bass_kernel_spmd

F32 = mybir.dt.float32
F16 = mybir.dt.float16
BF16 = mybir.dt.bfloat16

N_CORES = 8
BATCH, H, W = 128, 512, 512
IMGS_PER_CORE = BATCH // N_CORES      # 16
ROWS_PER_CORE = IMGS_PER_CORE * H     # 8192
BLK = 128
NBLK = H // BLK                       # 4
K = 11
PAD = K // 2                          # 5
ZH = K                                # zero head width
WT = ZH + PAD + W + PAD               # 533 segment width
XP0 = ZH                              # xp offset within segment (11)
X0 = ZH + PAD                         # x offset within segment (16)
FLAT = NBLK * WT                      # 2132
SCLEN = FLAT - ZH                     # 2121 scan steps
KH = 32                               # halo row-group size


def _band_matrices(dtype=np.float16):
    r = np.arange(BLK)
    bm_mid = (np.abs(r[:, None] - r[None, :]) <= PAD).astype(dtype)
    bm_top = bm_mid.copy()
    for rr in range(PAD):
        bm_top[0, rr] += dtype(PAD - rr)
    bm_bot = bm_mid.copy()
    for rr in range(BLK - PAD, BLK):
        bm_bot[BLK - 1, rr] += dtype(rr - (BLK - PAD - 1))
    # halo prev: weight rows are prev-segment partitions 96..127 (rel 0..31);
    # partition 96+p is image row (seg base - 32 + p); nonzero for p>=27:
    # row k = -32+p affects output r iff |r - k| <= 5 -> r <= p - 27.
    bhp = np.zeros((BLK, BLK), dtype=dtype)
    for p in range(BLK - PAD, BLK):
        bhp[p, 0:p - (BLK - PAD) + 1] = 1.0
    # halo next: partitions 0..31 of next segment = image rows 128+p;
    # affects r iff r >= 123+p, for p in 0..4.
    bhn = np.zeros((BLK, BLK), dtype=dtype)
    for p in range(PAD):
        bhn[p, BLK - PAD + p:BLK] = 1.0
    idn = (-121.0 * np.eye(BLK)).astype(dtype)
    return {"bm_top": bm_top, "bm_mid": bm_mid, "bm_bot": bm_bot,
            "bhp": bhp, "bhn": bhn, "idn": idn}


def _build():
    nc = bacc.Bacc(None, target_bir_lowering=False, debug=False)
    x_d = nc.declare_dram_parameter("x", [ROWS_PER_CORE, W], F16, isOutput=False)
    shapes = {"bm_top": [BLK, BLK], "bm_mid": [BLK, BLK], "bm_bot": [BLK, BLK],
              "bhp": [BLK, BLK], "bhn": [BLK, BLK], "idn": [BLK, BLK]}
    consts = {nm: nc.declare_dram_parameter(nm, sh, F16, isOutput=False)
              for nm, sh in shapes.items()}
    out_d = nc.declare_dram_parameter("out", [ROWS_PER_CORE, W], BF16, isOutput=True)
    xv = x_d[:].rearrange("(i p q) c -> i p q c", p=NBLK, q=BLK)   # [16,4,128,512]
    ov = out_d[:].rearrange("(i p q) c -> i p q c", p=NBLK, q=BLK)

    with tile.TileContext(nc) as tc:
        with (
            tc.tile_pool(name="cpool", bufs=1) as cpool,
            tc.tile_pool(name="xin", bufs=4) as x_pool,
            tc.tile_pool(name="scr", bufs=4) as s_pool,
            tc.tile_pool(name="outp", bufs=3) as o_pool,
            tc.tile_pool(name="psum", bufs=6, space=bass.MemorySpace.PSUM) as ps_pool,
        ):
            ct = {}
            for nm, d in consts.items():
                t = cpool.tile(list(d.shape), F16, tag=nm)
                nc.sync.dma_start(t[:], d[:])
                ct[nm] = t
            bias_t = cpool.tile([BLK, 1], F32, tag="bias")
            nc.vector.memset(bias_t[:], -242.0)

            imgs = {}  # i -> (ximg, scr_img, oimg)

            def front_img(i):
                ximg = x_pool.tile([BLK, NBLK, WT], F16, tag="ximg")
                nc.sync.dma_start(
                    ximg[:, :, X0:X0 + W],
                    xv[i].rearrange("p q c -> q p c"))
                nc.gpsimd.memset(ximg[:, :, 0:ZH], 0.0)
                nc.gpsimd.tensor_copy(
                    ximg[:, :, XP0:X0],
                    ximg[:, :, X0:X0 + 1].to_broadcast((BLK, NBLK, PAD)))
                nc.gpsimd.tensor_copy(
                    ximg[:, :, X0 + W:WT],
                    ximg[:, :, X0 + W - 1:X0 + W].to_broadcast((BLK, NBLK, PAD)))
                flat = ximg[:].rearrange("q p c -> q (p c)")
                s = s_pool.tile([BLK, SCLEN], F16, tag="scr")
                nc.vector.tensor_tensor_scan(
                    s[:], flat[:, ZH:FLAT], flat[:, 0:SCLEN], 0.0,
                    op0=mybir.AluOpType.add, op1=mybir.AluOpType.subtract)
                oimg = o_pool.tile([BLK, NBLK, W], BF16, tag="oimg")
                imgs[i] = (ximg, s, oimg)

            def back_img(i):
                ximg, s, oimg = imgs.pop(i)
                for pos in range(NBLK):
                    # W11 for segment pos lives at s[:, pos*WT+10 : +512]
                    seg = pos * WT + (K - 1)
                    sfx = "top" if pos == 0 else ("bot" if pos == NBLK - 1 else "mid")
                    ps = ps_pool.tile([BLK, W], F32, tag="ps")
                    mms = [(ct["bm_" + sfx][:], s[:, seg:seg + W], None)]
                    mms.append((ct["idn"][:], ximg[:, pos, X0:X0 + W], None))
                    if pos > 0:
                        pseg = (pos - 1) * WT + (K - 1)
                        mms.append((ct["bhp"][:], s[:, pseg:pseg + W], None))
                    if pos < NBLK - 1:
                        nseg = (pos + 1) * WT + (K - 1)
                        mms.append((ct["bhn"][:], s[:, nseg:nseg + W], None))
                    for j, (lhsT, rhs, tp) in enumerate(mms):
                        nc.tensor.matmul(ps[:], lhsT, rhs,
                                         start=(j == 0), stop=(j == len(mms) - 1),
                                         tile_position=tp)
                    nc.scalar.activation(
                        oimg[:, pos, :], ps[:], mybir.ActivationFunctionType.Sign,
                        bias=bias_t[:], scale=1.0)
                nc.sync.dma_start(ov[i].rearrange("p q c -> q p c"), oimg[:])

            front_img(0)
            front_img(1)
            front_img(2)
            for i in range(IMGS_PER_CORE):
                back_img(i)
                if i + 3 < IMGS_PER_CORE:
                    front_img(i + 3)
    nc.compile()
    return nc


_NC_CACHE = None


def kernel(x: np.ndarray) -> np.ndarray:
    global _NC_CACHE
    x = np.asarray(x, dtype=np.float32)
    x16 = x.reshape(BATCH, H, W).astype(np.float16)

    consts = _band_matrices()
    if _NC_CACHE is None:
        _NC_CACHE = _build()
    nc = _NC_CACHE

    in_maps = []
    for c in range(N_CORES):
        shard = x16[c * IMGS_PER_CORE:(c + 1) * IMGS_PER_CORE].reshape(
            ROWS_PER_CORE, W)
        m = {"x": np.ascontiguousarray(shard)}
        m.update(consts)
        in_maps.append(m)
    res = run_bass_kernel_spmd(nc, in_maps, core_ids=list(range(N_CORES)))
    out = np.empty((BATCH, H, W), dtype=np.float32)
    for c in range(N_CORES):
        sgn = res.results[c]["out"].astype(np.float32)
        out[c * IMGS_PER_CORE:(c + 1) * IMGS_PER_CORE] = \
            ((sgn >= 0.0) * np.float32(255.0)).reshape(IMGS_PER_CORE, H, W)
    return out.reshape(BATCH, H, W, 1)
